# revision 1
# baseline (speedup 1.0000x reference)
"""BiaffineAttention TRN2 kernel.

Full-input contract: kernel(**inputs) takes the unsharded reference inputs
(hidden_states [16,512,1024] f32 + MLP/bilinear params) and returns the full
arc_scores [16,512,512] f32.

Strategy:
- Data-parallel over batch across 8 NeuronCores (2 batches/core).
- All on-chip compute is feature-major (arc/hidden on partitions): every
  matmul contracts over partitions, no on-chip transposes.
- The whole dep-side second linear layer is eliminated algebraically:
      scores = (Hh@Wf + bf) @ (Hd@w2d^T + b2d)^T + bb
             = Hh @ G @ Hd^T + 1*(u.Hd^T) + (Hh.v)*1^T + c
  with G = Wf@w2d, u = bf@w2d, v = Wf@b2d, c = bf.b2d + bb (all folded on
  the host in float64, Wf = w2h^T@Wb, bf = b2h@Wb).  The rank-1 terms ride
  for free: u becomes the bias of the A = Hh@G epilogue, (Hh.v + c) lands in
  arc pad column 500 of A (G column 500 = v, bias col = c), and Hd's pad
  row 500 is forced to 1.0 via its layer-1 bias.  Device GEMMs per core:
  2x L1 (K=1024), A (K=512), scores (K=512) = 98304 PE cycles (was 114688).
- Inputs are packed into one striped DRAM tensor TIN[k, xT-b0|w1h|xT-b1|w1d]
  so the input stream needs only ~1.5 descriptor generations per L1 k-step,
  and the first k-tile lands in two parallel half-stripe DMAs.
- Matmuls run in float16 (10-bit mantissa, 2-byte operands at full PE rate).
"""

import sys

if "/opt/trn_rl_repo" not in sys.path:
    sys.path.insert(0, "/opt/trn_rl_repo")

import numpy as np

import concourse.bacc as bacc
import concourse.mybir as mybir
import concourse.tile as tile
from concourse.bass_utils import run_bass_kernel_spmd

N_CORES = 8
BATCH = 16
SEQ = 512
HIDDEN = 1024
ARC = 500
ARC_P = 512  # arc padded to a multiple of 128

P = 128
B_PER_CORE = BATCH // N_CORES  # 2
R = B_PER_CORE * SEQ  # 1024 rows per core
HK = HIDDEN // P  # 8 hidden k-tiles
AK = ARC_P // P  # 4 arc tiles
CW = 4 * SEQ  # TIN stripe width: xT-b0 | w1h | xT-b1 | w1d

F32 = mybir.dt.float32
F16 = mybir.dt.float16
AF = mybir.ActivationFunctionType
ADD = mybir.AluOpType.add
MAX = mybir.AluOpType.max

_CACHE = {}
_DEFAULTS = {"warm_n": 200, "s1_rings": "ysy", "jh_rings": "yg"}


def _cfg(name, default=None):
    return _CACHE.get(name, _DEFAULTS.get(name, default))


def _emit(nc, tc, aps, loop_n=0):
    import contextlib

    ctx = contextlib.ExitStack()
    with ctx:
        cpool = ctx.enter_context(tc.tile_pool(name="const", bufs=1))
        apool = ctx.enter_context(tc.tile_pool(name="acts", bufs=1))
        pspool = ctx.enter_context(tc.tile_pool(name="psum", bufs=8, space="PSUM"))
        opool = ctx.enter_context(tc.tile_pool(name="outs", bufs=8))

        # ---- resident SBUF tensors
        xw = cpool.tile([P, HK, CW], F16, tag="xw")  # xT-b0 | w1h | xT-b1 | w1d
        g = cpool.tile([P, AK, ARC_P], F16, tag="g")  # Gaug (arc1 parts, arc2)
        biases = cpool.tile([P, 3 * AK], F32, tag="biases")
        b1h = biases[:, 0 * AK : 1 * AK]
        uB = biases[:, 1 * AK : 2 * AK]
        b1d = biases[:, 2 * AK : 3 * AK]

        h1h = apool.tile([P, AK, R], F16, tag="h1h")
        h1d = apool.tile([P, AK, R], F16, tag="h1d")
        aT = apool.tile([P, AK, R], F16, tag="aT")

        # ---- PE warm-up: a chain of tiny self-contained matmuls keeps the
        # Tensor engine "busy" from ~0.7us so the real matmuls (whose first
        # operands land at ~3.6us) are issued past the p-state ramp window
        # and run at full clock.  The chain ends right around data arrival.
        warm_n = int(_cfg("warm_n", 225))
        if warm_n:
            wdum = cpool.tile([P, P], F16, tag="wdum")
            nc.vector.memset(wdum[:], 0.0)
            wps = pspool.tile([P, SEQ], F32, tag="ps", name="warm")
            for _ in range(warm_n):
                nc.tensor.matmul(wps[:, 0:16], wdum[:], wdum[:, 0:16], start=True, stop=True)

        # ---- input DMA schedule.  TIN stripe k = [xT-b0 | w1h | w1d | xT-b1]
        # for hidden k-tile k.  P0 consumes only cols 0:3*SEQ, so the head
        # stream sends those (1092ns/stripe vs 1704ns of PE work per stripe)
        # in consumption order; xT-b1 follows as one bulk transfer and the
        # Gaug weights last (needed only by the A phase).
        tin = aps["tin"].rearrange("(ko p) c -> p ko c", p=P)
        nc.sync.dma_start(xw[:, 0, 0 : SEQ + 2 * P], tin[:, 0, 0 : SEQ + 2 * P])
        nc.gpsimd.dma_start(xw[:, 0, SEQ + 2 * P : 3 * SEQ], tin[:, 0, SEQ + 2 * P : 3 * SEQ])
        nc.scalar.dma_start(xw[:, 1, 0 : 2 * SEQ], tin[:, 1, 0 : 2 * SEQ])
        nc.sync.dma_start(xw[:, 1, 2 * SEQ : 3 * SEQ], tin[:, 1, 2 * SEQ : 3 * SEQ])
        for k in range(2, HK):
            nc.sync.dma_start(xw[:, k], tin[:, k])
        nc.gpsimd.dma_start(biases[:], aps["biasesL"])
        nc.gpsimd.dma_start(xw[:, 0:2, 3 * SEQ : CW], tin[:, 0:2, 3 * SEQ : CW])
        nc.sync.dma_start(g[:], aps["g"].rearrange("(ko p) a -> p ko a", p=P))

        def l1_block(k, woff, rs, pss):
            # pss[m] += w1[:, k, m-slice].T @ xT[:, k, rs]
            for m in range(AK):
                nc.tensor.matmul(
                    pss[m][:],
                    xw[:, k, woff + m * P : woff + (m + 1) * P],
                    xw[:, k, rs],
                    start=(k == 0),
                    stop=(k == HK - 1),
                )

        def l1_phase(rc):
            # both L1s for batch-half rc, interleaved per k so the head
            # phase needs only one arriving stripe per 8 matmuls
            rs = slice(0, SEQ) if rc == 0 else slice(3 * SEQ, CW)
            ph = [pspool.tile([P, SEQ], F32, tag="ps", name=f"l1h_{rc}_{m}") for m in range(AK)]
            pd = [pspool.tile([P, SEQ], F32, tag="ps", name=f"l1d_{rc}_{m}") for m in range(AK)]
            for k in range(HK - 2):
                l1_block(k, SEQ, rs, ph)
                l1_block(k, 2 * SEQ, rs, pd)
            # finish the h side first: its epilogues overlap the trailing
            # d blocks, so the A phase starts with zero seam
            l1_block(HK - 2, SEQ, rs, ph)
            l1_block(HK - 1, SEQ, rs, ph)
            l1_block(HK - 2, 2 * SEQ, rs, pd)
            l1_block(HK - 1, 2 * SEQ, rs, pd)
            os = slice(rc * SEQ, (rc + 1) * SEQ)
            # h-epilogues split ACT/DVE so all four finish ~1.3us after the
            # last h matmul (the A-phase consumes them k2-by-k2)
            for m in range(AK):
                if m % 2 == 0:
                    nc.scalar.activation(h1h[:, m, os], ph[m][:], AF.Relu, bias=b1h[:, m : m + 1])
                else:
                    nc.vector.tensor_tensor(h1h[:, m, os], ph[m][:], b1h[:, m : m + 1].to_broadcast((P, SEQ)), ADD)
                    nc.vector.tensor_scalar_max(h1h[:, m, os], h1h[:, m, os], 0.0)
            # d-epilogues all on ACT (needed one A-phase later)
            for m in range(AK):
                nc.scalar.activation(h1d[:, m, os], pd[m][:], AF.Relu, bias=b1d[:, m : m + 1])

        def a_phase(rc):
            # aT[:, m, rc] = (Hh @ Gaug).T + u  (bias add per arc2 partition)
            rs = slice(rc * SEQ, (rc + 1) * SEQ)
            for m in range(AK):
                ps = pspool.tile([P, SEQ], F32, tag="ps", name=f"a_{rc}_{m}")
                for k2 in range(AK):
                    nc.tensor.matmul(
                        ps[:],
                        g[:, k2, m * P : (m + 1) * P],
                        h1h[:, k2, rs],
                        start=(k2 == 0),
                        stop=(k2 == AK - 1),
                    )
                # alternate DVE/ACT so the last tile's epilogue lands early
                if m % 2 == 0:
                    nc.vector.tensor_tensor(aT[:, m, rs], ps[:], uB[:, m : m + 1].to_broadcast((P, SEQ)), ADD)
                else:
                    nc.scalar.activation(aT[:, m, rs], ps[:], AF.Identity, bias=uB[:, m : m + 1])

        def s_phase(rc, last=False):
            # scores[rc][i-block] = aT-slice.T @ h1d ; i-outer so each output
            # tile drains (copy + DMA) while the next accumulates.  Early
            # tiles go out via the SWDGE ring (separate descriptor-gen unit);
            # the final tile of the kernel is j-split so its copy + DMA chain
            # after the last matmul is as short as possible.
            rs = slice(rc * SEQ, (rc + 1) * SEQ)
            H = SEQ // 2
            for i in range(AK):
                if last and i == AK - 1:
                    # j-split the final tile [448 | 64]; the first piece's DMA
                    # descriptor-gen goes to the otherwise-idle SWDGE unit so
                    # the last piece's HWDGE gen starts the moment its copy
                    # lands, on a ring whose sequencer is parked waiting on it
                    jsp = int(_cfg("jh_split", SEQ - 64))
                    for jh, (j0, j1) in enumerate(((0, jsp), (jsp, SEQ))):
                        w = j1 - j0
                        ps = pspool.tile([P, SEQ], F32, tag="ps", name=f"s_{rc}_{i}_{jh}")
                        js = slice(rc * SEQ + j0, rc * SEQ + j1)
                        for k2 in range(AK):
                            nc.tensor.matmul(
                                ps[:, 0:w],
                                aT[:, k2, rc * SEQ + i * P : rc * SEQ + (i + 1) * P],
                                h1d[:, k2, js],
                                start=(k2 == 0),
                                stop=(k2 == AK - 1),
                            )
                        ot = opool.tile([P, w], F16, tag=f"scout_{jh}")
                        if jh == 0:
                            nc.scalar.activation(ot[:], ps[:, 0:w], AF.Identity)
                        else:
                            nc.vector.tensor_copy(ot[:], ps[:, 0:w])
                        jr = _cfg("jh_rings", "gy")
                        eng = {"y": nc.sync, "s": nc.scalar, "g": nc.gpsimd}[jr[jh]]
                        eng.dma_start(
                            aps["scores"][rc, i * P : (i + 1) * P, j0:j1],
                            ot[:],
                        )
                    continue
                ps = pspool.tile([P, SEQ], F32, tag="ps", name=f"s_{rc}_{i}")
                for k2 in range(AK):
                    nc.tensor.matmul(
                        ps[:],
                        aT[:, k2, rc * SEQ + i * P : rc * SEQ + (i + 1) * P],
                        h1d[:, k2, rs],
                        start=(k2 == 0),
                        stop=(k2 == AK - 1),
                    )
                ot = opool.tile([P, SEQ], F16, tag="scout")
                if i % 2 == 0:
                    nc.vector.tensor_copy(ot[:], ps[:])
                else:
                    nc.scalar.activation(ot[:], ps[:], AF.Identity)
                if last:
                    rings = _cfg("s1_rings", "ysg")  # rings for i0,i1,i2
                    eng = {"y": nc.sync, "s": nc.scalar, "g": nc.gpsimd}[rings[i]]
                else:
                    eng = nc.sync if i % 2 == 0 else nc.scalar
                eng.dma_start(aps["scores"][rc, i * P : (i + 1) * P, :], ot[:])

        if loop_n:
            hints = _cfg("loop_hints", ())
            if hints == "all":
                hints = tuple(
                    mybir.EngineType(e) for e in ("PE", "Activation", "DVE", "SP", "Pool")
                )
            loop_cm = tc.For_i(0, loop_n, 1, hint_engines=hints)
        else:
            loop_cm = contextlib.nullcontext()
        if _cfg("tiny_body", False) and loop_n:
            with loop_cm:
                tb = apool.tile([P, 16], F32, tag="tinybody")
                nc.vector.tensor_copy(tb[:], biases[:, 0:16])
            return
        with loop_cm:
            l1_phase(0)
            a_phase(0)
            # first k-step of P1's head half fills the aT-epilogue seam
            rs1 = slice(3 * SEQ, CW)
            os1 = slice(SEQ, 2 * SEQ)
            ph1 = [pspool.tile([P, SEQ], F32, tag="ps", name=f"l1h_1_{m}") for m in range(AK)]
            l1_block(0, SEQ, rs1, ph1)
            s_phase(0)
            for k in range(1, HK):
                l1_block(k, SEQ, rs1, ph1)
            for m in range(AK):
                if m % 2 == 0:
                    nc.scalar.activation(h1h[:, m, os1], ph1[m][:], AF.Relu, bias=b1h[:, m : m + 1])
                else:
                    nc.vector.tensor_tensor(h1h[:, m, os1], ph1[m][:], b1h[:, m : m + 1].to_broadcast((P, SEQ)), ADD)
                    nc.vector.tensor_scalar_max(h1h[:, m, os1], h1h[:, m, os1], 0.0)
            pd1 = [pspool.tile([P, SEQ], F32, tag="ps", name=f"l1d_1_{m}") for m in range(AK)]
            for k in range(HK):
                l1_block(k, 2 * SEQ, rs1, pd1)
            for m in range(AK):
                nc.scalar.activation(h1d[:, m, os1], pd1[m][:], AF.Relu, bias=b1d[:, m : m + 1])
            a_phase(1)
            s_phase(1, last=True)


def _build(loop_n=0):
    key = ("nc", loop_n, _cfg("loop_hints", ()), _cfg("tiny_body", False),
           _cfg("warm_n", 225), _cfg("s1_rings", "ysg"), _cfg("jh_rings", "gy"),
           _cfg("jh_split", SEQ - 64))
    if key in _CACHE:
        return _CACHE[key]
    nc = bacc.Bacc("TRN2", target_bir_lowering=False, debug=False, num_devices=N_CORES)

    def dram(name, shape, dt):
        return nc.dram_tensor(name, shape, dt, kind="ExternalInput").ap()

    aps = {
        "tin": dram("tin", [HIDDEN, CW], F16),
        "g": dram("g", [ARC_P, ARC_P], F16),
        "biasesL": dram("biasesL", [P, 3 * AK], F32),
        "scores": nc.dram_tensor(
            "scores", [B_PER_CORE, SEQ, SEQ], F16, kind="ExternalOutput"
        ).ap(),
    }
    with tile.TileContext(nc) as tc:
        _emit(nc, tc, aps, loop_n=loop_n)
    nc.compile()
    _CACHE[key] = nc
    return nc


def _bias_layout(b):
    """[<=512] -> [128, AK] with arc index = col*128 + partition."""
    bp = np.zeros(ARC_P, np.float32)
    b = np.asarray(b, np.float32)
    bp[: b.shape[0]] = b
    return np.ascontiguousarray(bp.reshape(AK, P).T)


def _prep_shared(w1h, b1h, w2h, b2h, w1d, b1d, w2d, b2d, Wb, bb):
    f8 = np.float64
    w2h, b2h, w2d, b2d, Wb = (np.asarray(a, f8) for a in (w2h, b2h, w2d, b2d, Wb))
    bb0 = float(np.asarray(bb).reshape(-1)[0])
    Wf = w2h.T @ Wb  # [arc1, arc2]
    bf = b2h @ Wb  # [arc2]
    G = Wf @ w2d  # [arc1, arcd]
    u = bf @ w2d  # [arcd]
    v = Wf @ b2d  # [arc1]
    c = float(bf @ b2d) + bb0

    Gaug = np.zeros((ARC_P, ARC_P), np.float32)
    Gaug[:ARC, :ARC] = G
    Gaug[:ARC, ARC] = v
    u_aug = np.zeros(ARC_P, np.float64)
    u_aug[:ARC] = u
    u_aug[ARC] = c
    b1d_aug = np.zeros(ARC_P, np.float64)
    b1d_aug[:ARC] = np.asarray(b1d, f8)
    b1d_aug[ARC] = 1.0  # Hd pad column 500 = relu(0*x + 1) = 1

    def padT(w):
        out = np.zeros((HIDDEN, ARC_P), np.float32)
        wt = np.asarray(w, f8).T
        out[: wt.shape[0], : wt.shape[1]] = wt
        return out.astype(np.float16)

    return {
        "w1hT": padT(w1h),
        "w1dT": padT(w1d),
        "g": Gaug.astype(np.float16),
        "biasesL": np.concatenate(
            [
                _bias_layout(b1h),
                _bias_layout(u_aug.astype(np.float32)),
                _bias_layout(b1d_aug.astype(np.float32)),
            ],
            axis=1,
        ),
    }


def kernel(hidden_states, w1h, b1h, w2h, b2h, w1d, b1d, w2d, b2d, Wb, bb):
    import time

    nc = _build(loop_n=int(_cfg("loop_n", 0)))
    shared = _prep_shared(w1h, b1h, w2h, b2h, w1d, b1d, w2d, b2d, Wb, bb)
    x = np.asarray(hidden_states, np.float32)
    in_maps = []
    for c in range(N_CORES):
        xc = x[c * B_PER_CORE : (c + 1) * B_PER_CORE].reshape(R, HIDDEN)
        xT = np.ascontiguousarray(xc.T).astype(np.float16)  # [HIDDEN, R]
        tin = np.empty((HIDDEN, CW), np.float16)
        tin[:, 0:SEQ] = xT[:, 0:SEQ]
        tin[:, SEQ : 2 * SEQ] = shared["w1hT"]
        tin[:, 2 * SEQ : 3 * SEQ] = shared["w1dT"]
        tin[:, 3 * SEQ : CW] = xT[:, SEQ:R]
        in_maps.append({"tin": tin, "g": shared["g"], "biasesL": shared["biasesL"]})
    t0 = time.perf_counter()
    res = run_bass_kernel_spmd(nc, in_maps, core_ids=list(range(N_CORES)))
    _CACHE["last_run_seconds"] = time.perf_counter() - t0
    out = np.empty((BATCH, SEQ, SEQ), np.float32)
    for c in range(N_CORES):
        out[c * B_PER_CORE : (c + 1) * B_PER_CORE] = np.asarray(res.results[c]["scores"], np.float32)
    return out


def _selftest():
    rng = np.random.default_rng(0)
    s_h = 1.0 / np.sqrt(HIDDEN)
    s_a = 1.0 / np.sqrt(ARC)
    ins = {
        "hidden_states": rng.standard_normal((BATCH, SEQ, HIDDEN)).astype(np.float32),
        "w1h": rng.uniform(-s_h, s_h, (ARC, HIDDEN)).astype(np.float32),
        "b1h": rng.uniform(-s_h, s_h, (ARC,)).astype(np.float32),
        "w2h": rng.uniform(-s_a, s_a, (ARC, ARC)).astype(np.float32),
        "b2h": rng.uniform(-s_a, s_a, (ARC,)).astype(np.float32),
        "w1d": rng.uniform(-s_h, s_h, (ARC, HIDDEN)).astype(np.float32),
        "b1d": rng.uniform(-s_h, s_h, (ARC,)).astype(np.float32),
        "w2d": rng.uniform(-s_a, s_a, (ARC, ARC)).astype(np.float32),
        "b2d": rng.uniform(-s_a, s_a, (ARC,)).astype(np.float32),
        "Wb": rng.uniform(-s_a, s_a, (ARC, ARC)).astype(np.float32),
        "bb": rng.uniform(-s_a, s_a, (1,)).astype(np.float32),
    }
    out = kernel(**ins)

    def ref_mlp(x, w1, b1, w2, b2):
        return np.maximum(x @ w1.T + b1, 0.0) @ w2.T + b2

    head = ref_mlp(ins["hidden_states"], ins["w1h"], ins["b1h"], ins["w2h"], ins["b2h"])
    dep = ref_mlp(ins["hidden_states"], ins["w1d"], ins["b1d"], ins["w2d"], ins["b2d"])
    headW = head @ ins["Wb"]
    exp = np.einsum("bia,bja->bij", headW, dep) + ins["bb"][0]
    err = np.abs(out - exp)
    rel = err.max() / np.abs(exp).max()
    print(f"max abs err {err.max():.3e}  absmax-rel {rel:.3e}")
    print(f"run seconds: {_CACHE.get('last_run_seconds'):.3f}")


if __name__ == "__main__":
    _selftest()



# revision 16
# speedup vs baseline: 6.5620x; 6.5620x over previous
"""BiaffineAttention TRN2 kernel.

Full-input contract: kernel(**inputs) takes the unsharded reference inputs
(hidden_states [16,512,1024] f32 + MLP/bilinear params) and returns the full
arc_scores [16,512,512] f32.

Strategy:
- Data-parallel over batch across 8 NeuronCores (2 batches/core).
- All on-chip compute is feature-major (arc/hidden on partitions): every
  matmul contracts over partitions, no on-chip transposes.
- The whole dep-side second linear layer is eliminated algebraically:
      scores = (Hh@Wf + bf) @ (Hd@w2d^T + b2d)^T + bb
             = Hh @ G @ Hd^T + 1*(u.Hd^T) + (Hh.v)*1^T + c
  with G = Wf@w2d, u = bf@w2d, v = Wf@b2d, c = bf.b2d + bb (all folded on
  the host in float64, Wf = w2h^T@Wb, bf = b2h@Wb).  The rank-1 terms ride
  for free: u becomes the bias of the A = Hh@G epilogue, (Hh.v + c) lands in
  arc pad column 500 of A (G column 500 = v, bias col = c), and Hd's pad
  row 500 is forced to 1.0 via its layer-1 bias.  Device GEMMs per core:
  2x L1 (K=1024), A (K=512), scores (K=512) = 98304 PE cycles (was 114688).
- Inputs are packed into one striped DRAM tensor TIN[k, xT-b0|w1h|xT-b1|w1d]
  so the input stream needs only ~1.5 descriptor generations per L1 k-step,
  and the first k-tile lands in two parallel half-stripe DMAs.
- Matmuls run in bfloat16: HW-measured 242.8 ns per N=512 matmul vs 311.9 for
  float16 (fp16 streams at 3/4 column rate on TRN2), and the 8-bit mantissa
  still lands ~4.5e-3 relative error vs the 2e-2 gate.  In the 8-core
  sustained regime the PE throttles to ~310-400 ns/MM (power/thermal,
  drifts over minutes); bf16 keeps a consistent ~5% edge over fp16 there.
  The 192 matmuls per core are within 2.4% of the theoretical instruction
  floor (187.5) for this decomposition, and CoreSim shows the PE stream is
  gapless outside a ~3us drain tail, so the loop body is matmul-rate-bound.
"""

import sys

if "/opt/trn_rl_repo" not in sys.path:
    sys.path.insert(0, "/opt/trn_rl_repo")

import ml_dtypes
import numpy as np

BF16NP = ml_dtypes.bfloat16

import concourse.bacc as bacc
import concourse.mybir as mybir
import concourse.tile as tile
from concourse.bass_utils import run_bass_kernel_spmd

N_CORES = 8
BATCH = 16
SEQ = 512
HIDDEN = 1024
ARC = 500
ARC_P = 512  # arc padded to a multiple of 128

P = 128
B_PER_CORE = BATCH // N_CORES  # 2
R = B_PER_CORE * SEQ  # 1024 rows per core
HK = HIDDEN // P  # 8 hidden k-tiles
AK = ARC_P // P  # 4 arc tiles
CW = 4 * SEQ  # TIN stripe width: xT-b0 | w1h | xT-b1 | w1d

F32 = mybir.dt.float32
F16 = mybir.dt.float16
BF16 = mybir.dt.bfloat16
AF = mybir.ActivationFunctionType
ADD = mybir.AluOpType.add
MAX = mybir.AluOpType.max

_CACHE = {}
_DEFAULTS = {"warm_n": 200, "s1_rings": "ysy", "jh_rings": "yg"}


def _cfg(name, default=None):
    return _CACHE.get(name, _DEFAULTS.get(name, default))


def _emit(nc, tc, aps, loop_n=0):
    import contextlib

    MMDT = BF16 if _cfg("mm_dtype", "bf16") == "bf16" else F16
    ctx = contextlib.ExitStack()
    with ctx:
        cpool = ctx.enter_context(tc.tile_pool(name="const", bufs=1))
        apool = ctx.enter_context(tc.tile_pool(name="acts", bufs=1))
        pspool = ctx.enter_context(tc.tile_pool(name="psum", bufs=8, space="PSUM"))
        opool = ctx.enter_context(tc.tile_pool(name="outs", bufs=8))

        # ---- resident SBUF tensors
        xw = cpool.tile([P, HK, CW], MMDT, tag="xw")  # xT-b0 | w1h | xT-b1 | w1d
        g = cpool.tile([P, AK, ARC_P], MMDT, tag="g")  # Gaug (arc1 parts, arc2)
        biases = cpool.tile([P, 3 * AK], F32, tag="biases")
        b1h = biases[:, 0 * AK : 1 * AK]
        uB = biases[:, 1 * AK : 2 * AK]
        b1d = biases[:, 2 * AK : 3 * AK]

        h1h = apool.tile([P, AK, R], MMDT, tag="h1h")
        h1d = apool.tile([P, AK, R], MMDT, tag="h1d")
        aT = apool.tile([P, AK, R], MMDT, tag="aT")

        # ---- PE warm-up: a chain of tiny self-contained matmuls keeps the
        # Tensor engine "busy" from ~0.7us so the real matmuls (whose first
        # operands land at ~3.6us) are issued past the p-state ramp window
        # and run at full clock.  The chain ends right around data arrival.
        warm_n = int(_cfg("warm_n", 225))
        if warm_n:
            wdum = cpool.tile([P, P], MMDT, tag="wdum")
            nc.vector.memset(wdum[:], 0.0)
            wps = pspool.tile([P, SEQ], F32, tag="ps", name="warm")
            for _ in range(warm_n):
                nc.tensor.matmul(wps[:, 0:16], wdum[:], wdum[:, 0:16], start=True, stop=True)

        # ---- input DMA schedule.  TIN stripe k = [xT-b0 | w1h | w1d | xT-b1]
        # for hidden k-tile k.  P0 consumes only cols 0:3*SEQ, so the head
        # stream sends those (1092ns/stripe vs 1704ns of PE work per stripe)
        # in consumption order; xT-b1 follows as one bulk transfer and the
        # Gaug weights last (needed only by the A phase).
        tin = aps["tin"].rearrange("(ko p) c -> p ko c", p=P)
        nc.sync.dma_start(xw[:, 0, 0 : SEQ + 2 * P], tin[:, 0, 0 : SEQ + 2 * P])
        nc.gpsimd.dma_start(xw[:, 0, SEQ + 2 * P : 3 * SEQ], tin[:, 0, SEQ + 2 * P : 3 * SEQ])
        nc.scalar.dma_start(xw[:, 1, 0 : 2 * SEQ], tin[:, 1, 0 : 2 * SEQ])
        nc.sync.dma_start(xw[:, 1, 2 * SEQ : 3 * SEQ], tin[:, 1, 2 * SEQ : 3 * SEQ])
        for k in range(2, HK):
            nc.sync.dma_start(xw[:, k], tin[:, k])
        nc.gpsimd.dma_start(biases[:], aps["biasesL"])
        nc.gpsimd.dma_start(xw[:, 0:2, 3 * SEQ : CW], tin[:, 0:2, 3 * SEQ : CW])
        nc.sync.dma_start(g[:], aps["g"].rearrange("(ko p) a -> p ko a", p=P))

        def l1_block(k, woff, rs, pss):
            # pss[m] += w1[:, k, m-slice].T @ xT[:, k, rs]
            for m in range(AK):
                nc.tensor.matmul(
                    pss[m][:],
                    xw[:, k, woff + m * P : woff + (m + 1) * P],
                    xw[:, k, rs],
                    start=(k == 0),
                    stop=(k == HK - 1),
                )

        def l1_run(m, woff, rs, pss):
            # all HK k-steps for one (side, m) output tile: an 8-long
            # same-PSUM-bank matmul run (no bank cycling between MMs)
            for k in range(HK):
                nc.tensor.matmul(
                    pss[m][:],
                    xw[:, k, woff + m * P : woff + (m + 1) * P],
                    xw[:, k, rs],
                    start=(k == 0),
                    stop=(k == HK - 1),
                )

        def h_epilogue(m, os, ph):
            if m % 2 == 0:
                nc.scalar.activation(h1h[:, m, os], ph[m][:], AF.Relu, bias=b1h[:, m : m + 1])
            else:
                nc.vector.tensor_tensor(h1h[:, m, os], ph[m][:], b1h[:, m : m + 1].to_broadcast((P, SEQ)), ADD)
                nc.vector.tensor_scalar_max(h1h[:, m, os], h1h[:, m, os], 0.0)

        def l1_phase(rc):
            rs = slice(0, SEQ) if rc == 0 else slice(3 * SEQ, CW)
            ph = [pspool.tile([P, SEQ], F32, tag="ps", name=f"l1h_{rc}_{m}") for m in range(AK)]
            pd = [pspool.tile([P, SEQ], F32, tag="ps", name=f"l1d_{rc}_{m}") for m in range(AK)]
            os = slice(rc * SEQ, (rc + 1) * SEQ)
            if _cfg("l1_order", "k") == "m":
                # m-outer: same-bank runs of 8; each tile's epilogue issues
                # right after its run and overlaps the next run
                for m in range(AK):
                    l1_run(m, SEQ, rs, ph)
                    h_epilogue(m, os, ph)
                for m in range(AK):
                    l1_run(m, 2 * SEQ, rs, pd)
                    nc.scalar.activation(h1d[:, m, os], pd[m][:], AF.Relu, bias=b1d[:, m : m + 1])
                return
            # k-outer: both L1s for batch-half rc, interleaved per k so the
            # head phase needs only one arriving stripe per 8 matmuls
            for k in range(HK - 2):
                l1_block(k, SEQ, rs, ph)
                l1_block(k, 2 * SEQ, rs, pd)
            # finish the h side first: its epilogues overlap the trailing
            # d blocks, so the A phase starts with zero seam
            l1_block(HK - 2, SEQ, rs, ph)
            l1_block(HK - 1, SEQ, rs, ph)
            l1_block(HK - 2, 2 * SEQ, rs, pd)
            l1_block(HK - 1, 2 * SEQ, rs, pd)
            # h-epilogues split ACT/DVE so all four finish ~1.3us after the
            # last h matmul (the A-phase consumes them k2-by-k2)
            for m in range(AK):
                h_epilogue(m, os, ph)
            # d-epilogues all on ACT (needed one A-phase later)
            for m in range(AK):
                nc.scalar.activation(h1d[:, m, os], pd[m][:], AF.Relu, bias=b1d[:, m : m + 1])

        def a_phase(rc):
            # aT[:, m, rc] = (Hh @ Gaug).T + u  (bias add per arc2 partition)
            rs = slice(rc * SEQ, (rc + 1) * SEQ)
            for m in range(AK):
                ps = pspool.tile([P, SEQ], F32, tag="ps", name=f"a_{rc}_{m}")
                for k2 in range(AK):
                    nc.tensor.matmul(
                        ps[:],
                        g[:, k2, m * P : (m + 1) * P],
                        h1h[:, k2, rs],
                        start=(k2 == 0),
                        stop=(k2 == AK - 1),
                    )
                # alternate DVE/ACT so the last tile's epilogue lands early
                if m % 2 == 0:
                    nc.vector.tensor_tensor(aT[:, m, rs], ps[:], uB[:, m : m + 1].to_broadcast((P, SEQ)), ADD)
                else:
                    nc.scalar.activation(aT[:, m, rs], ps[:], AF.Identity, bias=uB[:, m : m + 1])

        def s_phase(rc, last=False):
            # scores[rc][i-block] = aT-slice.T @ h1d ; i-outer so each output
            # tile drains (copy + DMA) while the next accumulates.  Early
            # tiles go out via the SWDGE ring (separate descriptor-gen unit);
            # the final tile of the kernel is j-split so its copy + DMA chain
            # after the last matmul is as short as possible.
            rs = slice(rc * SEQ, (rc + 1) * SEQ)
            H = SEQ // 2
            obig = None
            if _cfg("out_batch", False):
                # one [128, 3-4, 512] SBUF staging tile per rc; a single
                # DMA replaces 3-4 per-tile DMAs (fixed cost amortized)
                nb = AK - 1 if last else AK
                obig = opool.tile([P, nb, SEQ], F16, tag=f"obig{rc}")
                sv = aps["scores"][rc].rearrange("(i p) j -> p i j", p=P)
                nbat = nb
                for i in range(AK):
                    if last and i == AK - 1:
                        break
                    ps = pspool.tile([P, SEQ], F32, tag="ps", name=f"s_{rc}_{i}")
                    for k2 in range(AK):
                        nc.tensor.matmul(
                            ps[:],
                            aT[:, k2, rc * SEQ + i * P : rc * SEQ + (i + 1) * P],
                            h1d[:, k2, rs],
                            start=(k2 == 0),
                            stop=(k2 == AK - 1),
                        )
                    if i % 2 == 0:
                        nc.vector.tensor_copy(obig[:, i, :], ps[:])
                    else:
                        nc.scalar.activation(obig[:, i, :], ps[:], AF.Identity)
                if not _cfg("skip_outdma", False):
                    nc.sync.dma_start(sv[:, 0:nbat, :], obig[:])
                if not last:
                    return
                # final tile: j-split as in the unbatched path
                i = AK - 1
                jsp = int(_cfg("jh_split", SEQ - 64))
                for jh, (j0, j1) in enumerate(((0, jsp), (jsp, SEQ))):
                    w = j1 - j0
                    ps = pspool.tile([P, SEQ], F32, tag="ps", name=f"s_{rc}_{i}_{jh}")
                    js = slice(rc * SEQ + j0, rc * SEQ + j1)
                    for k2 in range(AK):
                        nc.tensor.matmul(
                            ps[:, 0:w],
                            aT[:, k2, rc * SEQ + i * P : rc * SEQ + (i + 1) * P],
                            h1d[:, k2, js],
                            start=(k2 == 0),
                            stop=(k2 == AK - 1),
                        )
                    ot = opool.tile([P, w], F16, tag=f"scout_{jh}")
                    if jh == 0:
                        nc.scalar.activation(ot[:], ps[:, 0:w], AF.Identity)
                    else:
                        nc.vector.tensor_copy(ot[:], ps[:, 0:w])
                    jr = _cfg("jh_rings", "gy")
                    eng = {"y": nc.sync, "s": nc.scalar, "g": nc.gpsimd}[jr[jh]]
                    if not _cfg("skip_outdma", False):
                        eng.dma_start(
                            aps["scores"][rc, i * P : (i + 1) * P, j0:j1],
                            ot[:],
                        )
                return
            for i in range(AK):
                if last and i == AK - 1:
                    # j-split the final tile [448 | 64]; the first piece's DMA
                    # descriptor-gen goes to the otherwise-idle SWDGE unit so
                    # the last piece's HWDGE gen starts the moment its copy
                    # lands, on a ring whose sequencer is parked waiting on it
                    jsp = int(_cfg("jh_split", SEQ - 64))
                    for jh, (j0, j1) in enumerate(((0, jsp), (jsp, SEQ))):
                        w = j1 - j0
                        ps = pspool.tile([P, SEQ], F32, tag="ps", name=f"s_{rc}_{i}_{jh}")
                        js = slice(rc * SEQ + j0, rc * SEQ + j1)
                        for k2 in range(AK):
                            nc.tensor.matmul(
                                ps[:, 0:w],
                                aT[:, k2, rc * SEQ + i * P : rc * SEQ + (i + 1) * P],
                                h1d[:, k2, js],
                                start=(k2 == 0),
                                stop=(k2 == AK - 1),
                            )
                        ot = opool.tile([P, w], F16, tag=f"scout_{jh}")
                        if jh == 0:
                            nc.scalar.activation(ot[:], ps[:, 0:w], AF.Identity)
                        else:
                            nc.vector.tensor_copy(ot[:], ps[:, 0:w])
                        jr = _cfg("jh_rings", "gy")
                        eng = {"y": nc.sync, "s": nc.scalar, "g": nc.gpsimd}[jr[jh]]
                        if not _cfg("skip_outdma", False):
                            eng.dma_start(
                                aps["scores"][rc, i * P : (i + 1) * P, j0:j1],
                                ot[:],
                            )
                    continue
                ps = pspool.tile([P, SEQ], F32, tag="ps", name=f"s_{rc}_{i}")
                for k2 in range(AK):
                    nc.tensor.matmul(
                        ps[:],
                        aT[:, k2, rc * SEQ + i * P : rc * SEQ + (i + 1) * P],
                        h1d[:, k2, rs],
                        start=(k2 == 0),
                        stop=(k2 == AK - 1),
                    )
                ot = opool.tile([P, SEQ], F16, tag="scout")
                if i % 2 == 0:
                    nc.vector.tensor_copy(ot[:], ps[:])
                else:
                    nc.scalar.activation(ot[:], ps[:], AF.Identity)
                if last:
                    rings = _cfg("s1_rings", "ysg")  # rings for i0,i1,i2
                    eng = {"y": nc.sync, "s": nc.scalar, "g": nc.gpsimd}[rings[i]]
                else:
                    eng = nc.sync if i % 2 == 0 else nc.scalar
                if not _cfg("skip_outdma", False):
                    eng.dma_start(aps["scores"][rc, i * P : (i + 1) * P, :], ot[:])

        if loop_n:
            hints = _cfg("loop_hints", ())
            if hints == "all":
                hints = tuple(
                    mybir.EngineType(e) for e in ("PE", "Activation", "DVE", "SP", "Pool")
                )
            loop_cm = tc.For_i(0, loop_n, 1, hint_engines=hints)
        else:
            loop_cm = contextlib.nullcontext()
        if _cfg("tiny_body", False) and loop_n:
            with loop_cm:
                tb = apool.tile([P, 3 * AK], F32, tag="tinybody")
                nc.vector.tensor_copy(tb[:], biases[:])
            return
        body = _cfg("body", "full")
        if body != "full" and loop_n:
            # timing-ablation bodies (outputs are wrong; timing only)
            with loop_cm:
                l1_phase(0)
                if body in ("l1a",):
                    a_phase(0)
                rs1 = slice(3 * SEQ, CW)
                os1 = slice(SEQ, 2 * SEQ)
                ph1 = [pspool.tile([P, SEQ], F32, tag="ps", name=f"l1h_1_{m}") for m in range(AK)]
                for k in range(HK):
                    l1_block(k, SEQ, rs1, ph1)
                for m in range(AK):
                    h_epilogue(m, os1, ph1)
                pd1 = [pspool.tile([P, SEQ], F32, tag="ps", name=f"l1d_1_{m}") for m in range(AK)]
                for k in range(HK):
                    l1_block(k, 2 * SEQ, rs1, pd1)
                for m in range(AK):
                    nc.scalar.activation(h1d[:, m, os1], pd1[m][:], AF.Relu, bias=b1d[:, m : m + 1])
                if body in ("l1a",):
                    a_phase(1)
            return
        with loop_cm:
            l1_phase(0)
            a_phase(0)
            rs1 = slice(3 * SEQ, CW)
            os1 = slice(SEQ, 2 * SEQ)
            ph1 = [pspool.tile([P, SEQ], F32, tag="ps", name=f"l1h_1_{m}") for m in range(AK)]
            if _cfg("l1_order", "k") == "m":
                # first m-run of P1's head half fills the aT-epilogue seam
                l1_run(0, SEQ, rs1, ph1)
                s_phase(0)
                h_epilogue(0, os1, ph1)
                for m in range(1, AK):
                    l1_run(m, SEQ, rs1, ph1)
                    h_epilogue(m, os1, ph1)
                pd1 = [pspool.tile([P, SEQ], F32, tag="ps", name=f"l1d_1_{m}") for m in range(AK)]
                for m in range(AK):
                    l1_run(m, 2 * SEQ, rs1, pd1)
                    nc.scalar.activation(h1d[:, m, os1], pd1[m][:], AF.Relu, bias=b1d[:, m : m + 1])
            else:
                # first k-step of P1's head half fills the aT-epilogue seam
                l1_block(0, SEQ, rs1, ph1)
                s_phase(0)
                for k in range(1, HK):
                    l1_block(k, SEQ, rs1, ph1)
                for m in range(AK):
                    h_epilogue(m, os1, ph1)
                pd1 = [pspool.tile([P, SEQ], F32, tag="ps", name=f"l1d_1_{m}") for m in range(AK)]
                for k in range(HK):
                    l1_block(k, 2 * SEQ, rs1, pd1)
                for m in range(AK):
                    nc.scalar.activation(h1d[:, m, os1], pd1[m][:], AF.Relu, bias=b1d[:, m : m + 1])
            a_phase(1)
            s_phase(1, last=True)


def _build(loop_n=0):
    key = ("nc", loop_n, _cfg("loop_hints", ()), _cfg("tiny_body", False),
           _cfg("warm_n", 225), _cfg("s1_rings", "ysg"), _cfg("jh_rings", "gy"),
           _cfg("jh_split", SEQ - 64), _cfg("mm_dtype", "bf16"),
           _cfg("l1_order", "k"), _cfg("body", "full"),
           _cfg("skip_outdma", False), _cfg("out_batch", False))
    if key in _CACHE:
        return _CACHE[key]
    nc = bacc.Bacc("TRN2", target_bir_lowering=False, debug=False, num_devices=N_CORES)

    def dram(name, shape, dt):
        return nc.dram_tensor(name, shape, dt, kind="ExternalInput").ap()

    aps = {
        "tin": dram("tin", [HIDDEN, CW],
                    BF16 if _cfg("mm_dtype", "bf16") == "bf16" else F16),
        "g": dram("g", [ARC_P, ARC_P],
                  BF16 if _cfg("mm_dtype", "bf16") == "bf16" else F16),
        "biasesL": dram("biasesL", [P, 3 * AK], F32),
        "scores": nc.dram_tensor(
            "scores", [B_PER_CORE, SEQ, SEQ], F16, kind="ExternalOutput"
        ).ap(),
    }
    with tile.TileContext(nc) as tc:
        _emit(nc, tc, aps, loop_n=loop_n)
    nc.compile()
    _CACHE[key] = nc
    return nc


def _bias_layout(b):
    """[<=512] -> [128, AK] with arc index = col*128 + partition."""
    bp = np.zeros(ARC_P, np.float32)
    b = np.asarray(b, np.float32)
    bp[: b.shape[0]] = b
    return np.ascontiguousarray(bp.reshape(AK, P).T)


def _mmdt_np():
    return BF16NP if _cfg("mm_dtype", "bf16") == "bf16" else np.float16


def _prep_shared(w1h, b1h, w2h, b2h, w1d, b1d, w2d, b2d, Wb, bb):
    f8 = np.float64
    w2h, b2h, w2d, b2d, Wb = (np.asarray(a, f8) for a in (w2h, b2h, w2d, b2d, Wb))
    bb0 = float(np.asarray(bb).reshape(-1)[0])
    Wf = w2h.T @ Wb  # [arc1, arc2]
    bf = b2h @ Wb  # [arc2]
    G = Wf @ w2d  # [arc1, arcd]
    u = bf @ w2d  # [arcd]
    v = Wf @ b2d  # [arc1]
    c = float(bf @ b2d) + bb0

    Gaug = np.zeros((ARC_P, ARC_P), np.float32)
    Gaug[:ARC, :ARC] = G
    Gaug[:ARC, ARC] = v
    u_aug = np.zeros(ARC_P, np.float64)
    u_aug[:ARC] = u
    u_aug[ARC] = c
    b1d_aug = np.zeros(ARC_P, np.float64)
    b1d_aug[:ARC] = np.asarray(b1d, f8)
    b1d_aug[ARC] = 1.0  # Hd pad column 500 = relu(0*x + 1) = 1

    def padT(w):
        out = np.zeros((HIDDEN, ARC_P), np.float32)
        wt = np.asarray(w, f8).T
        out[: wt.shape[0], : wt.shape[1]] = wt
        return out.astype(_mmdt_np())

    return {
        "w1hT": padT(w1h),
        "w1dT": padT(w1d),
        "g": Gaug.astype(_mmdt_np()),
        "biasesL": np.concatenate(
            [
                _bias_layout(b1h),
                _bias_layout(u_aug.astype(np.float32)),
                _bias_layout(b1d_aug.astype(np.float32)),
            ],
            axis=1,
        ),
    }


def kernel(hidden_states, w1h, b1h, w2h, b2h, w1d, b1d, w2d, b2d, Wb, bb):
    import time

    nc = _build(loop_n=int(_cfg("loop_n", 0)))
    shared = _prep_shared(w1h, b1h, w2h, b2h, w1d, b1d, w2d, b2d, Wb, bb)
    x = np.asarray(hidden_states, np.float32)
    in_maps = []
    for c in range(N_CORES):
        xc = x[c * B_PER_CORE : (c + 1) * B_PER_CORE].reshape(R, HIDDEN)
        xT = np.ascontiguousarray(xc.T).astype(_mmdt_np())  # [HIDDEN, R]
        tin = np.empty((HIDDEN, CW), _mmdt_np())
        tin[:, 0:SEQ] = xT[:, 0:SEQ]
        tin[:, SEQ : 2 * SEQ] = shared["w1hT"]
        tin[:, 2 * SEQ : 3 * SEQ] = shared["w1dT"]
        tin[:, 3 * SEQ : CW] = xT[:, SEQ:R]
        in_maps.append({"tin": tin, "g": shared["g"], "biasesL": shared["biasesL"]})
    t0 = time.perf_counter()
    res = run_bass_kernel_spmd(nc, in_maps, core_ids=list(range(N_CORES)))
    _CACHE["last_run_seconds"] = time.perf_counter() - t0
    out = np.empty((BATCH, SEQ, SEQ), np.float32)
    for c in range(N_CORES):
        out[c * B_PER_CORE : (c + 1) * B_PER_CORE] = np.asarray(res.results[c]["scores"], np.float32)
    return out


def _selftest():
    rng = np.random.default_rng(0)
    s_h = 1.0 / np.sqrt(HIDDEN)
    s_a = 1.0 / np.sqrt(ARC)
    ins = {
        "hidden_states": rng.standard_normal((BATCH, SEQ, HIDDEN)).astype(np.float32),
        "w1h": rng.uniform(-s_h, s_h, (ARC, HIDDEN)).astype(np.float32),
        "b1h": rng.uniform(-s_h, s_h, (ARC,)).astype(np.float32),
        "w2h": rng.uniform(-s_a, s_a, (ARC, ARC)).astype(np.float32),
        "b2h": rng.uniform(-s_a, s_a, (ARC,)).astype(np.float32),
        "w1d": rng.uniform(-s_h, s_h, (ARC, HIDDEN)).astype(np.float32),
        "b1d": rng.uniform(-s_h, s_h, (ARC,)).astype(np.float32),
        "w2d": rng.uniform(-s_a, s_a, (ARC, ARC)).astype(np.float32),
        "b2d": rng.uniform(-s_a, s_a, (ARC,)).astype(np.float32),
        "Wb": rng.uniform(-s_a, s_a, (ARC, ARC)).astype(np.float32),
        "bb": rng.uniform(-s_a, s_a, (1,)).astype(np.float32),
    }
    out = kernel(**ins)

    def ref_mlp(x, w1, b1, w2, b2):
        return np.maximum(x @ w1.T + b1, 0.0) @ w2.T + b2

    head = ref_mlp(ins["hidden_states"], ins["w1h"], ins["b1h"], ins["w2h"], ins["b2h"])
    dep = ref_mlp(ins["hidden_states"], ins["w1d"], ins["b1d"], ins["w2d"], ins["b2d"])
    headW = head @ ins["Wb"]
    exp = np.einsum("bia,bja->bij", headW, dep) + ins["bb"][0]
    err = np.abs(out - exp)
    rel = err.max() / np.abs(exp).max()
    print(f"max abs err {err.max():.3e}  absmax-rel {rel:.3e}")
    print(f"run seconds: {_CACHE.get('last_run_seconds'):.3f}")


if __name__ == "__main__":
    _selftest()

